# revision 3
# baseline (speedup 1.0000x reference)
"""MaskLinear kernel for 8x TRN2 NeuronCores.

Computes out[m,d] = sum_n weight[n] * masks[m,n] * x[n,d] + bias
 (= (masks * weight) @ x + bias), with x:[100000,256], masks:[64,100000].

Strategy: shard the contraction axis N across 8 cores. Each core gets a
12500-row slice (zero-padded to 12544 = 98*128), computes a partial
[64,256] via 98 accumulating matmuls (lhsT = maskT*weight chunk [128,64],
rhs = x chunk [128,256]), and the host sums the 8 partials + bias.

Host-side layout: masks is transposed to [N,64] with weight prepended as
column 0, so each device chunk loads as one contiguous per-partition DMA
and a single tensor_scalar multiply folds the weight in on-device.
"""

import numpy as np

import concourse.bacc as bacc
import concourse.mybir as mybir
from concourse import tile
from concourse.bass_utils import run_bass_kernel_spmd

N_CORES = 8
N = 100000
D = 256
M = 64
NS = N // N_CORES          # 12500 rows per shard
CHUNK = 128                # matmul contraction tile (partition dim)
C = -(-NS // CHUNK)        # 98 chunks
NP = C * CHUNK             # 12544 padded rows per shard
B = 14                     # chunks per DMA group
G = C // B                 # 7 groups
MW = M + 1                 # weight col + 64 mask cols

assert G * B == C

_STATE = {}


def _build_nc(mm_dtype="f32r"):
    nc = bacc.Bacc("TRN2", target_bir_lowering=False, debug=False,
                   num_devices=N_CORES)

    f32 = mybir.dt.float32
    if mm_dtype == "f32r":
        mm_dt = mybir.dt.float32r
    elif mm_dtype == "f32":
        mm_dt = f32
    else:
        raise ValueError(mm_dtype)

    # np view of both f32 and f32r is float32, so host arrays stay f32.
    xs = nc.dram_tensor("xs", [NP, D], mm_dt, kind="ExternalInput")
    ms = nc.dram_tensor("ms", [NP, MW], mm_dt, kind="ExternalInput")
    out = nc.dram_tensor("out", [M, D], f32, kind="ExternalOutput")

    # Row (g*128 + p)*B + b -> group g, partition p, sub-chunk b: each
    # partition's slice of a group is one contiguous B*row run in DRAM.
    xr = xs[:, :].rearrange("(g p b) d -> g p (b d)", p=CHUNK, b=B)
    mr = ms[:, :].rearrange("(g p b) j -> g p (b j)", p=CHUNK, b=B)

    with tile.TileContext(nc) as tc:
        with (
            tc.tile_pool(name="xp", bufs=3) as xp,
            tc.tile_pool(name="mp", bufs=3) as mp,
            tc.tile_pool(name="wp", bufs=4) as wp,
            tc.tile_pool(name="pp", bufs=1, space="PSUM") as pp,
            tc.tile_pool(name="op", bufs=1) as op,
        ):
            psum = pp.tile([M, D], f32)
            for g in range(G):
                xt = xp.tile([CHUNK, B * D], mm_dt, tag="xt")
                mt = mp.tile([CHUNK, B * MW], mm_dt, tag="mt")
                nc.sync.dma_start(xt[:], xr[g])
                nc.sync.dma_start(mt[:], mr[g])
                for b in range(B):
                    c = g * B + b
                    wm = wp.tile([CHUNK, M], mm_dt, tag="wm")
                    nc.vector.tensor_scalar_mul(
                        wm[:],
                        mt[:, b * MW + 1:(b + 1) * MW],
                        mt[:, b * MW:b * MW + 1].bitcast(f32),
                    )
                    nc.tensor.matmul(
                        psum[:],
                        wm[:],
                        xt[:, b * D:(b + 1) * D],
                        start=(c == 0),
                        stop=(c == C - 1),
                    )
            osb = op.tile([M, D], f32)
            nc.scalar.copy(osb[:], psum[:])
            nc.sync.dma_start(out[:, :], osb[:])
    nc.compile()
    return nc


def _get_nc():
    if "nc" not in _STATE:
        _STATE["nc"] = _build_nc()
    return _STATE["nc"]


def _shard_inputs(x, masks, weight):
    x = np.ascontiguousarray(np.asarray(x, dtype=np.float32))
    masks = np.asarray(masks, dtype=np.float32)
    weight = np.asarray(weight, dtype=np.float32)

    in_maps = []
    for s in range(N_CORES):
        lo = s * NS
        hi = lo + NS
        xs = np.zeros((NP, D), np.float32)
        xs[:NS] = x[lo:hi]
        ms = np.zeros((NP, MW), np.float32)
        ms[:NS, 0] = weight[lo:hi]
        ms[:NS, 1:] = masks[:, lo:hi].T
        in_maps.append({"xs": xs, "ms": ms})
    return in_maps


def _run(x, masks, weight, bias, **run_kwargs):
    in_maps = _shard_inputs(x, masks, weight)
    res = run_bass_kernel_spmd(
        _get_nc(), in_maps, core_ids=list(range(N_CORES)), **run_kwargs
    )
    parts = np.stack([r["out"] for r in res.results])
    out = parts.sum(axis=0) + np.asarray(bias, dtype=np.float32)
    return out.astype(np.float32), res


def kernel(x, masks, weight, bias):
    out, _ = _run(x, masks, weight, bias)
    return out


# revision 5
# speedup vs baseline: 1.6168x; 1.6168x over previous
"""MaskLinear kernel for 8x TRN2 NeuronCores.

Computes out[m,d] = sum_n weight[n] * masks[m,n] * x[n,d] + bias
 (= (masks * weight) @ x + bias), with x:[100000,256], masks:[64,100000].

Strategy: shard the contraction axis N across 8 cores. Each core gets a
12500-row slice (zero-padded to 12544 = 98*128), computes a partial
[64,256] via 98 accumulating matmuls (lhsT = maskT*weight chunk [128,64],
rhs = x chunk [128,256]), and the host sums the 8 partials + bias.

Host-side layout: masks is transposed to [N,64] with weight prepended as
column 0, so each device group loads with per-partition-contiguous DMAs
and one broadcast tensor_mul per group folds the weight in on-device.
"""

import numpy as np

import concourse.bacc as bacc
import concourse.mybir as mybir
from concourse import tile
from concourse.bass_utils import run_bass_kernel_spmd

N_CORES = 8
N = 100000
D = 256
M = 64
NS = N // N_CORES          # 12500 rows per shard
CHUNK = 128                # matmul contraction tile (partition dim)
C = -(-NS // CHUNK)        # 98 chunks
NP = C * CHUNK             # 12544 padded rows per shard
B = 14                     # chunks per DMA group
G = C // B                 # 7 groups
MW = M + 1                 # weight col + 64 mask cols

assert G * B == C

MODE = "f32r"              # "f32r" (fp32 traffic) or "bf16" (half traffic)

_STATE = {}


def _np_dtype(mode):
    if mode == "bf16":
        import ml_dtypes
        return np.dtype(ml_dtypes.bfloat16)
    return np.dtype(np.float32)


def _build_nc(mode):
    nc = bacc.Bacc("TRN2", target_bir_lowering=False, debug=False,
                   num_devices=N_CORES)

    f32 = mybir.dt.float32
    if mode == "f32r":
        mm_dt = mybir.dt.float32r
        dve_view = f32     # DVE ALU ops reject f32r operands; bitcast to f32
    elif mode == "bf16":
        mm_dt = mybir.dt.bfloat16
        dve_view = mybir.dt.bfloat16
    else:
        raise ValueError(mode)

    xs = nc.dram_tensor("xs", [NP, D], mm_dt, kind="ExternalInput")
    ms = nc.dram_tensor("ms", [NP, MW], mm_dt, kind="ExternalInput")
    out = nc.dram_tensor("out", [M, D], f32, kind="ExternalOutput")

    # Row (g*128 + p)*B + b -> group g, partition p, sub-chunk b: each
    # partition's slice of a group is one contiguous B-row run in DRAM.
    xr = xs[:, :].rearrange("(g p b) d -> g p (b d)", p=CHUNK, b=B)
    mr = ms[:, :].rearrange("(g p b) j -> g p (b j)", p=CHUNK, b=B)

    with tile.TileContext(nc) as tc:
        with (
            tc.tile_pool(name="xp", bufs=3) as xp,
            tc.tile_pool(name="mp", bufs=3) as mp,
            tc.tile_pool(name="wp", bufs=3) as wp,
            tc.tile_pool(name="pp", bufs=1, space="PSUM") as pp,
            tc.tile_pool(name="op", bufs=1) as op,
        ):
            psum = pp.tile([M, D], f32)
            for g in range(G):
                xt = xp.tile([CHUNK, B * D], mm_dt, tag="xt")
                mt = mp.tile([CHUNK, B * MW], mm_dt, tag="mt")
                nc.sync.dma_start(xt[:], xr[g])
                nc.sync.dma_start(mt[:], mr[g])

                # wm[:, b, :] = mt[:, b, 1:] * mt[:, b, 0] for all b at once.
                # Inputs viewed as f32 (DVE ALU rejects f32r operands); the
                # out AP keeps the f32r type so the BIR verifier accepts the
                # downstream f32r matmul.
                wm = wp.tile([CHUNK, B * M], mm_dt, tag="wm")
                mt3 = mt[:].bitcast(dve_view).rearrange("p (b j) -> p b j", b=B)
                wm3 = wm[:].rearrange("p (b j) -> p b j", b=B)
                nc.vector.tensor_mul(
                    wm3,
                    mt3[:, :, 1:MW],
                    mt3[:, :, 0:1].broadcast_to((CHUNK, B, M)),
                )
                for b in range(B):
                    c = g * B + b
                    nc.tensor.matmul(
                        psum[:],
                        wm[:, b * M:(b + 1) * M],
                        xt[:, b * D:(b + 1) * D],
                        start=(c == 0),
                        stop=(c == C - 1),
                    )
            osb = op.tile([M, D], f32)
            nc.scalar.copy(osb[:], psum[:])
            nc.sync.dma_start(out[:, :], osb[:])
    nc.compile()
    return nc


def _get_nc(mode):
    key = "nc_" + mode
    if key not in _STATE:
        _STATE[key] = _build_nc(mode)
    return _STATE[key]


def _shard_inputs(x, masks, weight, mode):
    dt = _np_dtype(mode)
    x = np.asarray(x, dtype=np.float32)
    masks = np.asarray(masks, dtype=np.float32)
    weight = np.asarray(weight, dtype=np.float32)

    in_maps = []
    for s in range(N_CORES):
        lo = s * NS
        hi = lo + NS
        xs = np.zeros((NP, D), dt)
        xs[:NS] = x[lo:hi].astype(dt, copy=False)
        ms = np.zeros((NP, MW), dt)
        ms[:NS, 0] = weight[lo:hi].astype(dt, copy=False)
        ms[:NS, 1:] = masks[:, lo:hi].T.astype(dt, copy=False)
        in_maps.append({"xs": xs, "ms": ms})
    return in_maps


def _run(x, masks, weight, bias, mode=MODE, **run_kwargs):
    in_maps = _shard_inputs(x, masks, weight, mode)
    res = run_bass_kernel_spmd(
        _get_nc(mode), in_maps, core_ids=list(range(N_CORES)), **run_kwargs
    )
    parts = np.stack([r["out"] for r in res.results])
    out = parts.sum(axis=0) + np.asarray(bias, dtype=np.float32)
    return out.astype(np.float32), res


def kernel(x, masks, weight, bias):
    out, _ = _run(x, masks, weight, bias)
    return out
